# revision 9
# baseline (speedup 1.0000x reference)
"""MaxPool3d (kernel=3, stride=2, padding=1) on Trainium2, 8 NeuronCores.

Input  x: (2, 32, 128, 128, 128) f32  ->  Output: (2, 32, 64, 64, 64) f32.

Sharding: the 64 (b, c) slices are data-parallel; each of the 8 cores gets 8
slices, processed as 4 slice-pairs (a pair packs 2 slices into the 128 SBUF
partitions: partition 64*s + d//2 holds depth rows 2k/2k+1 of slice s in the
free-dim parity slot).

Per-core algorithm (separable max pooling H -> W -> D), fp16 intermediates
(max() commutes with the monotone f32->fp16 rounding, so the result equals
fp16(true max): rel err <= 2^-11; fp16 gives the DVE its 2x_1P perf mode on
every unit-stride max):
  - SWDGE (gpsimd) cast-loads stream x f32 from HBM into fp16 SBUF tiles,
    hc h-rows per chunk, every input byte read exactly once.  The stream
    runs gap-free at full HBM rate; six tile buffers let it run far ahead
    so the DVE is gated by cumulative bytes, not per-chunk completion
    jitter.
  - H pool on the free h axis at 2x: F[j] = max(r[2j], r[2j+1], r[2j-1]);
    the r[2j-1] tap of the chunk's first row comes from the previous
    chunk's last row (still resident), so chunks need no overlap.
  - W pool on the free w axis (stride-2 sources, 1x): G = max over w window.
  - D pool across partitions: out = max(G_E, G_O, G_O shifted down one
    partition); the shift is a small SBUF->SBUF DMA on the sync ring; rows
    0/64 are covered by an idempotent duplicate copy.
  - Output is stored as fp16 (scalar ring) and upcast to f32 on the host.

Engine roles keep every DMA-issuing stream single-purpose so the in-order
HWDGE/SWDGE FIFOs never block a load behind a late dependency: gpsimd =
loads only, sync = partition shifts only, scalar = stores only.
"""

import os
import sys

sys.path.insert(0, "/opt/trn_rl_repo")

import numpy as np

# Shapes (hardcoded per problem spec)
B, C, D, H, W = 2, 32, 128, 128, 128
OD, OH, OW = 64, 64, 64
N_CORES = 8
SLICES_PER_CORE = (B * C) // N_CORES  # 8
PAIRS = SLICES_PER_CORE // 2  # 4
HC = 32  # h rows pooled per chunk
NCH = H // HC  # 4

_cache = {}


def _build():
    import concourse.mybir as mybir
    from concourse import bacc
    from concourse.tile import TileContext

    f32 = mybir.dt.float32
    f16 = mybir.dt.float16
    nc = bacc.Bacc()
    x_ext = nc.declare_dram_parameter(
        "x_shard", [SLICES_PER_CORE, D, H, W], f32, isOutput=False
    )
    y_ext = nc.declare_dram_parameter(
        "y_shard", [SLICES_PER_CORE, OD, OH, OW], f16, isOutput=True
    )

    with TileContext(nc) as tc:
        with (
            tc.tile_pool(name="hpool", bufs=6) as hpool,
            tc.tile_pool(name="fpool", bufs=2) as fpool,
            tc.tile_pool(name="gpool", bufs=2) as gpool,
            tc.tile_pool(name="ypool", bufs=2) as ypool,
        ):
            for p in range(PAIRS):
                s0 = 2 * p
                xh_prev = None
                for c in range(NCH):
                    h0 = HC * c
                    oh0 = h0 // 2
                    ohc = HC // 2  # 16 output rows per chunk
                    # ---- SWDGE cast-load: hc rows, both parities ----
                    xh = hpool.tile([128, 2, HC, W], f16, name="xh", tag="xh")
                    for par in (0, 1):
                        nc.gpsimd.dma_start(
                            out=xh[:, par : par + 1, :, :],
                            in_=x_ext[s0 : s0 + 2, par : D : 2, h0 : h0 + HC, :],
                        )
                    # ---- H pool (free axis, fp16 2x): F[j] = max of rows
                    # 2j, 2j+1, 2j-1; the 2j-1 tap of j=0 lives in the
                    # previous chunk (h = -1 pad for the first chunk).
                    Ft = fpool.tile([128, 2, HC // 2, W], f16, name="Ft", tag="Ft")
                    nc.vector.tensor_max(
                        out=Ft,
                        in0=xh[:, :, 0:HC:2, :],
                        in1=xh[:, :, 1:HC:2, :],
                    )
                    nc.vector.tensor_max(
                        out=Ft[:, :, 1:, :],
                        in0=Ft[:, :, 1:, :],
                        in1=xh[:, :, 1 : HC - 2 : 2, :],
                    )
                    if xh_prev is not None:
                        nc.vector.tensor_max(
                            out=Ft[:, :, 0:1, :],
                            in0=Ft[:, :, 0:1, :],
                            in1=xh_prev[:, :, HC - 1 : HC, :],
                        )
                    xh_prev = xh
                    # ---- W pool (free axis, stride-2 sources, 1x) ----
                    Gt = gpool.tile([128, 2, HC // 2, OW], f16, name="Gt", tag="Gt")
                    nc.vector.tensor_max(
                        out=Gt,
                        in0=Ft[:, :, :, 0:W:2],
                        in1=Ft[:, :, :, 1:W:2],
                    )
                    nc.vector.tensor_max(
                        out=Gt[:, :, :, 1:OW],
                        in0=Gt[:, :, :, 1:OW],
                        in1=Ft[:, :, :, 1 : W - 2 : 2],
                    )
                    # ---- D pool (partition axis) ----
                    # shifted copy of the odd slab (sync ring): Gs[k] =
                    # G_O[k-1]; rows 0/64 get the idempotent unshifted value.
                    Gs = gpool.tile([128, 1, HC // 2, OW], f16, name="Gs", tag="Gs")
                    nc.sync.dma_start(out=Gs[1:64], in_=Gt[0:63, 1:2, :, :])
                    nc.sync.dma_start(out=Gs[65:128], in_=Gt[64:127, 1:2, :, :])
                    nc.sync.dma_start(out=Gs[0:65:64], in_=Gt[0:65:64, 1:2, :, :])
                    Yh = ypool.tile([128, 1, HC // 2, OW], f16, name="Yh", tag="Yh")
                    nc.vector.tensor_max(
                        out=Yh, in0=Gt[:, 0:1, :, :], in1=Gt[:, 1:2, :, :]
                    )
                    nc.vector.tensor_max(out=Yh, in0=Yh, in1=Gs)
                    # ---- store fp16 output rows (scalar ring) ----
                    nc.scalar.dma_start(
                        out=y_ext[s0 : s0 + 2, :, oh0 : oh0 + ohc, :], in_=Yh
                    )
    nc.compile()
    return nc


def _get_nc():
    if "nc" not in _cache:
        _cache["nc"] = _build()
    return _cache["nc"]


def run(x: np.ndarray, **spmd_kwargs):
    """Run the SPMD kernel; returns the BassKernelResults (for tracing)."""
    from concourse.bass_utils import run_bass_kernel_spmd

    nc = _get_nc()
    xs = np.ascontiguousarray(x, dtype=np.float32).reshape(B * C, D, H, W)
    in_maps = [
        {"x_shard": np.ascontiguousarray(xs[SLICES_PER_CORE * i : SLICES_PER_CORE * (i + 1)])}
        for i in range(N_CORES)
    ]
    return run_bass_kernel_spmd(nc, in_maps, list(range(N_CORES)), **spmd_kwargs)


def kernel(x: np.ndarray) -> np.ndarray:
    res = run(x)
    out = np.stack([res.results[i]["y_shard"] for i in range(N_CORES)])
    return out.reshape(B, C, OD, OH, OW).astype(np.float32)
